# revision 1
# baseline (speedup 1.0000x reference)
"""Weighted-MSE loss kernel (nn_LossWithEuler) for 8 Trainium2 NeuronCores.

loss = mean(weight[b] * (inp[d,b] - label[d,b])^2)
  weight[b]  = attr_w[b] * angle_w[b]
  attr_w[b]  = sum_j (attribute[j,b]==1) * (sum(attribute_num)/attribute_num[j])
  angle_w[b] = sum_j (1 - cos(ea[j,b])) = sum_j 2*sin(ea[j,b]/2)^2

Sharding: batch axis B=131072 split across 8 cores (16384 each). Each core's
shard is host-transposed to (16384, 136) so that b sits on SBUF partitions:
partition p holds b in [p*128, (p+1)*128) as 128 contiguous rows of 136 floats.
Per-core partial sums [128,1] are combined on the host.
"""

import sys
import numpy as np

D = 136
B = 131072
N_CORES = 8
BS = B // N_CORES  # 16384 b's per core
P = 128            # SBUF partitions
Q = BS // P        # 128 b's per partition
NCHUNK = 8         # main-loop chunks over the free dim
CB = Q // NCHUNK   # 32 b's per chunk
CF = CB * D        # 4352 free elements per chunk

_program = None


def _build_program():
    try:
        import concourse.bass as bass
    except ImportError:
        sys.path.insert(0, "/opt/trn_rl_repo")
        import concourse.bass as bass
    from concourse import bacc, mybir, tile

    f32 = mybir.dt.float32
    i32 = mybir.dt.int32
    AF = mybir.ActivationFunctionType
    OP = mybir.AluOpType
    AX = mybir.AxisListType

    nc = bacc.Bacc("TRN2", target_bir_lowering=False, debug=False,
                   num_devices=N_CORES)

    # inp and label shards stacked on the host: data[0]=inp.T, data[1]=label.T
    data = nc.dram_tensor("data", (2, BS, D), f32, kind="ExternalInput")
    ea = nc.dram_tensor("ea", (3, BS), f32, kind="ExternalInput")
    attr = nc.dram_tensor("attr", (6, BS), i32, kind="ExternalInput")
    # attribute_num replicated to all 128 partitions on the host; inv_freq is
    # computed redundantly per partition (avoids an on-device broadcast).
    anum = nc.dram_tensor("anum", (P, 6), f32, kind="ExternalInput")
    out = nc.dram_tensor("out", (P, 1), f32, kind="ExternalOutput")

    # (2, BS, D) viewed as [128 partitions, tensor(2), Q*D free]: per chunk a
    # single DMA loads the inp AND label slices (one wait on the consumer).
    data_v = data.ap().rearrange("t (p q) d -> p t (q d)", p=P)

    with tile.TileContext(nc) as tc:
        with tc.tile_pool(name="const", bufs=1) as cpool, \
             tc.tile_pool(name="main", bufs=4) as mpool, \
             tc.tile_pool(name="diffp", bufs=3) as dpool:
            # ---- main-loop data DMAs: chunk 0 and 1 issued first so the
            # critical-path HBM stream starts as early as possible.
            def chunk_dma(c):
                t = mpool.tile([P, 2 * CF], f32, tag="data")
                nc.sync.dma_start(
                    t[:].rearrange("p (t f) -> p t f", t=2),
                    data_v[:, :, c * CF:(c + 1) * CF],
                )
                return t

            dts = {0: chunk_dma(0), 1: chunk_dma(1)}

            # ---- small weight-input DMAs (queued behind chunks 0/1) ----
            a_sb = cpool.tile([P, 6], f32)
            nc.sync.dma_start(a_sb[:], anum.ap())
            attr_i = cpool.tile([P, 6 * Q], i32)
            nc.sync.dma_start(
                attr_i[:].rearrange("p (j q) -> p j q", q=Q),
                attr.ap().rearrange("j (p q) -> p j q", q=Q),
            )
            ea_sb = cpool.tile([P, 3 * Q], f32)
            nc.sync.dma_start(
                ea_sb[:].rearrange("p (j q) -> p j q", q=Q),
                ea.ap().rearrange("j (p q) -> p j q", q=Q),
            )

            # ---- main loop, software-pipelined: DVE stream is
            # sub_0, sub_1, tr_0, sub_2, tr_1, ... so each chunk's ACT square
            # overlaps the next chunk's subtract instead of serializing.
            colsq = cpool.tile([P, Q], f32)
            dfs = {}
            for c in range(NCHUNK):
                if c + 2 < NCHUNK:
                    dts[c + 2] = chunk_dma(c + 2)
                dt_ = dts[c]
                df = dpool.tile([P, CF], f32, tag="diff")
                dfs[c] = df
                # Logical-priority ticks: force the scheduler to place
                # chunk c-1's reduce AFTER chunk c's subtract in the DVE
                # stream, so the ACT square overlaps the next subtract.
                with tc.tile_wait_until(0.004 * c):
                    nc.vector.tensor_sub(df[:], dt_[:, 0:CF], dt_[:, CF:2 * CF])
                if c >= 1:
                    with tc.tile_wait_until(0.004 * c):
                        nc.scalar.activation(
                            dfs[c - 1][:], dfs[c - 1][:], AF.Square)
                    with tc.tile_wait_until(0.004 * c + 0.002):
                        nc.vector.tensor_reduce(
                            colsq[:, (c - 1) * CB:c * CB],
                            dfs[c - 1][:].rearrange("p (b d) -> p b d", d=D),
                            axis=AX.X, op=OP.add,
                        )
            c = NCHUNK - 1
            nc.scalar.activation(dfs[c][:], dfs[c][:], AF.Square)
            nc.vector.tensor_reduce(
                colsq[:, c * CB:(c + 1) * CB],
                dfs[c][:].rearrange("p (b d) -> p b d", d=D),
                axis=AX.X, op=OP.add,
            )

            # ---- weight computation (inputs landed long ago; these small ops
            # run at the tail of each engine's stream).
            # inverse-frequency: ivb[p,j] = sum(anum)/anum[j]
            tot = cpool.tile([P, 1], f32)
            nc.vector.tensor_reduce(tot[:], a_sb[:], axis=AX.X, op=OP.add)
            rec = cpool.tile([P, 6], f32)
            nc.vector.reciprocal(rec[:], a_sb[:])
            ivb = cpool.tile([P, 6], f32)
            nc.vector.tensor_scalar_mul(ivb[:], rec[:], tot[:, 0:1])
            # attr_w[p,q] = sum_j attr[j, p*128+q] * iv[j]
            attr_f = cpool.tile([P, 6 * Q], f32)
            nc.vector.tensor_copy(attr_f[:], attr_i[:])
            aw0 = cpool.tile([P, Q], f32)
            aw1 = cpool.tile([P, Q], f32)
            nc.vector.tensor_scalar_mul(aw0[:], attr_f[:, 0:Q], ivb[:, 0:1])
            cur, nxt = aw0, aw1
            for j in range(1, 6):
                nc.vector.scalar_tensor_tensor(
                    nxt[:], attr_f[:, j * Q:(j + 1) * Q], ivb[:, j:j + 1],
                    cur[:], op0=OP.mult, op1=OP.add,
                )
                cur, nxt = nxt, cur
            aw = cur
            # angle_w[p,q] = 2 * sum_j sin(ea[j, p*128+q]/2)^2
            sinh_sb = cpool.tile([P, 3 * Q], f32)
            nc.scalar.activation(sinh_sb[:], ea_sb[:], AF.Sin, bias=0.0, scale=0.5)
            ssq = cpool.tile([P, 3 * Q], f32)
            nc.vector.tensor_mul(ssq[:], sinh_sb[:], sinh_sb[:])
            angle = cpool.tile([P, Q], f32)
            nc.vector.tensor_reduce(
                angle[:], ssq[:].rearrange("p (j q) -> p q j", q=Q),
                axis=AX.X, op=OP.add,
            )
            # weight[p,q] = (2*angle) * attr_w
            w_sb = cpool.tile([P, Q], f32)
            nc.vector.scalar_tensor_tensor(
                w_sb[:], angle[:], 2.0, aw[:], op0=OP.mult, op1=OP.mult,
            )

            # ---- partial[p] = sum_q colsq[p,q] * weight[p,q] ----
            scr = cpool.tile([P, Q], f32)
            part = cpool.tile([P, 1], f32)
            nc.vector.tensor_mul(scr[:], colsq[:], w_sb[:])
            nc.vector.tensor_reduce(part[:], scr[:], axis=AX.X, op=OP.add)
            nc.sync.dma_start(out.ap(), part[:])

    nc.compile()
    return nc


def _get_program():
    global _program
    if _program is None:
        _program = _build_program()
    return _program


def _make_in_maps(inp, label, ea, attribute, attribute_num):
    inp = np.asarray(inp, dtype=np.float32)
    label = np.asarray(label, dtype=np.float32)
    ea = np.asarray(ea, dtype=np.float32)
    attribute = np.asarray(attribute, dtype=np.int32)
    anum = np.tile(np.asarray(attribute_num, dtype=np.float32).reshape(1, 6),
                   (P, 1))
    in_maps = []
    for c in range(N_CORES):
        s = slice(c * BS, (c + 1) * BS)
        dat = np.empty((2, BS, D), dtype=np.float32)
        dat[0] = inp[:, s].T
        dat[1] = label[:, s].T
        in_maps.append({
            "data": dat,
            "ea": np.ascontiguousarray(ea[:, s]),
            "attr": np.ascontiguousarray(attribute[:, s]),
            "anum": anum,
        })
    return in_maps


def run(inputs, trace=False, trace_cores=None):
    """Run on hardware; returns (result_scalar, BassKernelResults)."""
    try:
        from concourse.bass_utils import run_bass_kernel_spmd
    except ImportError:
        sys.path.insert(0, "/opt/trn_rl_repo")
        from concourse.bass_utils import run_bass_kernel_spmd
    nc = _get_program()
    in_maps = _make_in_maps(**inputs)
    kwargs = {}
    if trace:
        kwargs["trace"] = True
        if trace_cores is not None:
            kwargs["trace_cores"] = trace_cores
    res = run_bass_kernel_spmd(nc, in_maps, core_ids=list(range(N_CORES)), **kwargs)
    total = 0.0
    for r in res.results:
        total += r["out"].astype(np.float64).sum()
    value = np.asarray(total / (D * B), dtype=np.float32)
    return value, res


def kernel(**inputs):
    value, _ = run(inputs)
    return value



# revision 2
# speedup vs baseline: 1.4913x; 1.4913x over previous
"""Weighted-MSE loss kernel (nn_LossWithEuler) for 8 Trainium2 NeuronCores.

loss = mean(weight[b] * (inp[d,b] - label[d,b])^2)
  weight[b]  = attr_w[b] * angle_w[b]
  attr_w[b]  = sum_j (attribute[j,b]==1) * (sum(attribute_num)/attribute_num[j])
  angle_w[b] = sum_j (1 - cos(ea[j,b]))

Sharding: batch axis B=131072 split across 8 cores (16384 each). Each core's
shard is host-transposed to (16384, 136) bf16 so that b sits on SBUF
partitions: partition p holds b in [p*128, (p+1)*128) as 128 contiguous rows
of 136 values. inp/label travel as bf16 (the 2e-2 rel-err budget is ~500x
above the bf16 pipeline error); ea/attribute stay f32/i32.

Schedule: the inp+label chunk stream runs on the qSP HWDGE queue
back-to-back; ea/attr/anum ride the qAct HWDGE queue so they never occupy
the main stream. DVE is software-pipelined (sub_c issued before
square_{c-1}/reduce_{c-1}) so it never idles waiting on ACT. The final
partition reduction happens on-device (GpSimd axis-C reduce) so the output
DMA is a single 4-byte descriptor instead of 128 of them.

USE_ACCUM variant: label is negated on the host and added into the inp tile
by the DMA itself (SWDGE accum_op=add on the qPool queue), removing the
subtract from DVE.
"""

import sys
import numpy as np

D = 136
B = 131072
N_CORES = 8
BS = B // N_CORES  # 16384 b's per core
P = 128            # SBUF partitions
Q = BS // P        # 128 b's per partition
NCHUNK = 16        # main-loop chunks over the free dim
CB = Q // NCHUNK   # b's per chunk
CF = CB * D        # free elements per chunk per tensor

USE_ACCUM = False  # DMA-accumulate -label into the inp tile (SWDGE)

_program = None


def _build_program():
    try:
        import concourse.bass as bass
    except ImportError:
        sys.path.insert(0, "/opt/trn_rl_repo")
        import concourse.bass as bass
    from concourse import bacc, mybir, tile

    f32 = mybir.dt.float32
    bf16 = mybir.dt.bfloat16
    i32 = mybir.dt.int32
    AF = mybir.ActivationFunctionType
    OP = mybir.AluOpType
    AX = mybir.AxisListType

    nc = bacc.Bacc("TRN2", target_bir_lowering=False, debug=False,
                   num_devices=N_CORES)

    # inp and label shards stacked on the host: data[0]=inp.T, data[1]=label.T
    # (label negated when USE_ACCUM).
    data = nc.dram_tensor("data", (2, BS, D), bf16, kind="ExternalInput")
    ea = nc.dram_tensor("ea", (3, BS), f32, kind="ExternalInput")
    attr = nc.dram_tensor("attr", (6, BS), i32, kind="ExternalInput")
    # attribute_num replicated to all 128 partitions on the host; inv_freq is
    # computed redundantly per partition (avoids an on-device broadcast).
    anum = nc.dram_tensor("anum", (P, 6), f32, kind="ExternalInput")
    out = nc.dram_tensor("out", (1, 1), f32, kind="ExternalOutput")

    # (2, BS, D) viewed as [128 partitions, tensor(2), Q*D free].
    data_v = data.ap().rearrange("t (p q) d -> p t (q d)", p=P)

    with tile.TileContext(nc) as tc:
        with tc.tile_pool(name="const", bufs=1) as cpool, \
             tc.tile_pool(name="main", bufs=8) as mpool:

            if USE_ACCUM:
                def chunk_dma(c):
                    t = mpool.tile([P, CF], bf16, tag="data")
                    nc.sync.dma_start(t[:], data_v[:, 0, c * CF:(c + 1) * CF])
                    nc.gpsimd.dma_start(
                        t[:], data_v[:, 1, c * CF:(c + 1) * CF],
                        accum_op=OP.add,
                    )
                    return t
            else:
                def chunk_dma(c):
                    t = mpool.tile([P, 2 * CF], bf16, tag="data")
                    nc.sync.dma_start(
                        t[:].rearrange("p (t f) -> p t f", t=2),
                        data_v[:, :, c * CF:(c + 1) * CF],
                    )
                    return t

            # main-loop chunk 0/1 DMAs issued first: critical-path stream.
            dts = {0: chunk_dma(0), 1: chunk_dma(1)}

            # ---- small weight-input DMAs on the qAct HWDGE queue (parallel
            # with the main stream; ACT has plenty of slack).
            a_sb = cpool.tile([P, 6], f32)
            nc.scalar.dma_start(a_sb[:], anum.ap())
            ea_sb = cpool.tile([P, 3 * Q], f32)
            nc.scalar.dma_start(
                ea_sb[:].rearrange("p (j q) -> p j q", q=Q),
                ea.ap().rearrange("j (p q) -> p j q", q=Q),
            )
            attr_i = cpool.tile([P, 6 * Q], i32)
            nc.scalar.dma_start(
                attr_i[:].rearrange("p (j q) -> p j q", q=Q),
                attr.ap().rearrange("j (p q) -> p j q", q=Q),
            )

            # ---- weight computation: runs in the chunk-0/1 DMA shadow.
            # angle_w first so ACT's sin (and its table load) happen early.
            sinh_sb = cpool.tile([P, 3 * Q], f32)
            nc.scalar.activation(sinh_sb[:], ea_sb[:], AF.Sin, bias=0.0,
                                 scale=0.5)
            ssq = cpool.tile([P, 3 * Q], f32)
            nc.vector.tensor_mul(ssq[:], sinh_sb[:], sinh_sb[:])
            angle = cpool.tile([P, Q], f32)
            nc.vector.tensor_reduce(
                angle[:], ssq[:].rearrange("p (j q) -> p q j", q=Q),
                axis=AX.X, op=OP.add,
            )
            # inverse-frequency: ivb[p,j] = sum(anum)/anum[j]
            tot = cpool.tile([P, 1], f32)
            nc.vector.tensor_reduce(tot[:], a_sb[:], axis=AX.X, op=OP.add)
            rec = cpool.tile([P, 6], f32)
            nc.vector.reciprocal(rec[:], a_sb[:])
            ivb = cpool.tile([P, 6], f32)
            nc.vector.tensor_scalar_mul(ivb[:], rec[:], tot[:, 0:1])
            # attr_w[p,q] = sum_j attr[j, p*128+q] * iv[j]
            attr_f = cpool.tile([P, 6 * Q], f32)
            nc.vector.tensor_copy(attr_f[:], attr_i[:])
            aw0 = cpool.tile([P, Q], f32)
            aw1 = cpool.tile([P, Q], f32)
            nc.vector.tensor_scalar_mul(aw0[:], attr_f[:, 0:Q], ivb[:, 0:1])
            cur, nxt = aw0, aw1
            for j in range(1, 6):
                nc.vector.scalar_tensor_tensor(
                    nxt[:], attr_f[:, j * Q:(j + 1) * Q], ivb[:, j:j + 1],
                    cur[:], op0=OP.mult, op1=OP.add,
                )
                cur, nxt = nxt, cur
            aw = cur
            # weight[p,q] = (2*angle) * attr_w
            w_sb = cpool.tile([P, Q], f32)
            nc.vector.scalar_tensor_tensor(
                w_sb[:], angle[:], 2.0, aw[:], op0=OP.mult, op1=OP.mult,
            )

            # ---- main loop, software-pipelined by emission order:
            # DVE stream is sub_0, sub_1, red_0, sub_2, red_1, ... so each
            # chunk's ACT square overlaps the next chunk's subtract.
            colsq = cpool.tile([P, Q], f32)

            def square_reduce(c):
                dt_ = dts.pop(c)
                nc.scalar.activation(dt_[:, 0:CF], dt_[:, 0:CF], AF.Square)
                nc.vector.tensor_reduce(
                    colsq[:, c * CB:(c + 1) * CB],
                    dt_[:, 0:CF].rearrange("p (b d) -> p b d", d=D),
                    axis=AX.X, op=OP.add,
                )

            for c in range(NCHUNK):
                if c + 2 < NCHUNK:
                    dts[c + 2] = chunk_dma(c + 2)
                if not USE_ACCUM:
                    dt_ = dts[c]
                    nc.vector.tensor_sub(
                        dt_[:, 0:CF], dt_[:, 0:CF], dt_[:, CF:2 * CF])
                if c >= 1:
                    square_reduce(c - 1)
            square_reduce(NCHUNK - 1)

            # ---- partial[p] = sum_q colsq[p,q] * weight[p,q], then the
            # cross-partition sum on GpSimd so the output is a single 4-byte
            # descriptor (a [P,1] store is 128 4-byte HBM RMW writes, ~8us).
            scr = cpool.tile([P, Q], f32)
            part = cpool.tile([P, 1], f32)
            nc.vector.tensor_mul(scr[:], colsq[:], w_sb[:])
            nc.vector.tensor_reduce(part[:], scr[:], axis=AX.X, op=OP.add)
            res = cpool.tile([1, 1], f32)
            nc.gpsimd.tensor_reduce(res[:], part[:], axis=AX.C, op=OP.add)
            nc.sync.dma_start(out.ap(), res[:])

    nc.compile()
    return nc


def _get_program():
    global _program
    if _program is None:
        _program = _build_program()
    return _program


def _make_in_maps(inp, label, ea, attribute, attribute_num):
    import ml_dtypes
    bf16 = ml_dtypes.bfloat16
    inp = np.asarray(inp, dtype=np.float32)
    label = np.asarray(label, dtype=np.float32)
    ea = np.asarray(ea, dtype=np.float32)
    attribute = np.asarray(attribute, dtype=np.int32)
    anum = np.tile(np.asarray(attribute_num, dtype=np.float32).reshape(1, 6),
                   (P, 1))
    in_maps = []
    for c in range(N_CORES):
        s = slice(c * BS, (c + 1) * BS)
        dat = np.empty((2, BS, D), dtype=bf16)
        dat[0] = inp[:, s].T.astype(bf16)
        lab = label[:, s].T
        dat[1] = (-lab if USE_ACCUM else lab).astype(bf16)
        in_maps.append({
            "data": dat,
            "ea": np.ascontiguousarray(ea[:, s]),
            "attr": np.ascontiguousarray(attribute[:, s]),
            "anum": anum,
        })
    return in_maps


def run(inputs, trace=False, trace_cores=None):
    """Run on hardware; returns (result_scalar, BassKernelResults)."""
    try:
        from concourse.bass_utils import run_bass_kernel_spmd
    except ImportError:
        sys.path.insert(0, "/opt/trn_rl_repo")
        from concourse.bass_utils import run_bass_kernel_spmd
    nc = _get_program()
    in_maps = _make_in_maps(**inputs)
    kwargs = {}
    if trace:
        kwargs["trace"] = True
        if trace_cores is not None:
            kwargs["trace_cores"] = trace_cores
    res = run_bass_kernel_spmd(nc, in_maps, core_ids=list(range(N_CORES)), **kwargs)
    total = 0.0
    for r in res.results:
        total += float(r["out"].astype(np.float64).sum())
    value = np.asarray(total / (D * B), dtype=np.float32)
    return value, res


def kernel(**inputs):
    value, _ = run(inputs)
    return value
